# revision 2
# baseline (speedup 1.0000x reference)
"""3-layer GCN encoder (GCNConv+BN+ReLU x3) on 8 Trainium2 NeuronCores.

Strategy (graph/data-parallel over destination nodes), v2 = bf16 pipeline:
  - Nodes padded 50000 -> 50176 = 8 * 6272; core c owns dst rows
    [c*6272, (c+1)*6272) = 49 blocks of 128.
  - All activations/weights/messages are bf16 (PSUM accumulates fp32);
    BN statistics and normalization math stay fp32.
  - Per layer l: each core computes its shard of H = (X @ W_l) * dinv
    channel-major on the PE, transposes to node-major bf16, AllGathers
    the full 50176 x 64 bf16 table.
  - The table is addressed as 25088 rows of 128 bf16 (= 256B = two
    nodes per row), so a single int16 index (src >> 1) covers the whole
    node range: no address-half split, and every gather descriptor is a
    256B transfer.
  - Edges (incl. self loops) are sorted by dst block and, within a
    block, partitioned by src parity. A tile of 128 even-src (odd-src)
    edges uses columns 0:64 (64:128) of the gathered pair-rows, so each
    tile needs ONE one-hot [slot -> dstrel] bf16 matmul to segment-sum
    messages into a [64ch x 128dst] PSUM accumulator; the result is
    scaled by dinv[dst] (per-src dinv is folded into the table).
  - BatchNorm: per-channel sum / sumsq over the local shard via ACT
    accum_out, AllReduce across cores, then one fused
    Relu(S*A + B) activation (A = gamma*rsqrt(var+eps), B = beta - mean*A).
    Conv biases are mathematically absorbed by BN's mean subtraction.
  - Host side does integer index preprocessing only (sort/partition/pad,
    degree counting, layout transposes); all FP math runs on device.
"""
import sys
sys.path.insert(0, "/opt/trn_rl_repo")
import numpy as np

import concourse.bass as bass
import concourse.mybir as mybir
import concourse.tile as tile
from concourse import library_config
from concourse.library_overlay import lower_extended_insts
from concourse.masks import make_identity

N = 50000
NPAD = 50176
NCORES = 8
SHARD = NPAD // NCORES          # 6272
NB = SHARD // 128               # 49 blocks per core
NPAIR = NPAD // 2               # 25088 pair-rows (int16-safe)
IN_C = 128
HID = 64
BN_EPS = 1e-5
F32 = mybir.dt.float32
BF16 = mybir.dt.bfloat16
I16 = mybir.dt.int16


def _split_multi_waits(nc, cap=1):
    """walrus in this toolchain accepts one sync wait per instruction;
    hoist extras onto standalone same-engine NOPs."""
    ctr = 0
    for func in nc.m.functions:
        for bb in func.blocks:
            new_insts = []
            for inst in bb.instructions:
                si = inst.sync_info
                if si is not None and len(si.on_wait) > cap:
                    waits = list(si.on_wait)
                    for w in waits[:-cap]:
                        ctr += 1
                        new_insts.append(mybir.InstNoOp(
                            name=f"waitsplit-{ctr}-{inst.name}",
                            sync_info=mybir.SyncInfo(on_wait=[w], on_update=[]),
                            bass_nofuse=True,
                            engine=inst.engine,
                        ))
                    inst.sync_info = mybir.SyncInfo(
                        on_wait=waits[-cap:], on_update=list(si.on_update))
                new_insts.append(inst)
            bb.instructions = new_insts
    return ctr


def _plan(TA, TB):
    """Group blocks in pairs; ONE gather call per group covering the
    paired blocks' even+odd tile segments contiguously.
    Returns (groups, SUMT): groups = list of dicts with
      blocks: tuple of block ids
      call:   (tile_start, ntiles)
      seg:    {(block, parity): (tile_start, ntiles)}
    Tile indices are global (into dstrel / idx column space)."""
    groups = []
    toff = 0
    b = 0
    while b < NB:
        blocks = tuple(range(b, min(b + 4, NB)))
        seg = {}
        call_start = toff
        for blk in blocks:
            seg[(blk, 0)] = (toff, TA[blk])
            toff += TA[blk]
            seg[(blk, 1)] = (toff, TB[blk])
            toff += TB[blk]
        groups.append({
            "blocks": blocks,
            "call": (call_start, toff - call_start),
            "seg": seg,
        })
        b += 4
    return groups, toff


def build_kernel(TA, TB, reps=1, do_gather=True, do_compute=True, do_ag=True):
    """TA/TB: per-block tile counts (len NB) for even-src / odd-src edges,
    uniform across cores (max over cores, baked into the program).
    reps>1 replicates the whole 3-layer body (timing instrument only)."""
    groups, SUMT = _plan(TA, TB)

    nc = bass.Bass(num_swdge_queues=4)
    xT_in = nc.dram_tensor("xT", [IN_C, SHARD], BF16, kind="ExternalInput")
    degbc_in = nc.dram_tensor("degbc", [HID, SHARD], F32, kind="ExternalInput")
    idx_in = nc.dram_tensor("idx", [128, SUMT * 8], I16, kind="ExternalInput")
    dstrel_in = nc.dram_tensor("dstrel", [128, SUMT], F32, kind="ExternalInput")
    iota_in = nc.dram_tensor("iota", [128, 128], BF16, kind="ExternalInput")
    w1_in = nc.dram_tensor("w1", [IN_C, HID], BF16, kind="ExternalInput")
    w2_in = nc.dram_tensor("w2", [HID, HID], BF16, kind="ExternalInput")
    w3_in = nc.dram_tensor("w3", [HID, HID], BF16, kind="ExternalInput")
    gb_in = nc.dram_tensor("gb", [HID, 6], F32, kind="ExternalInput")  # g1,be1,g2,be2,g3,be3
    out_t = nc.dram_tensor("outT", [HID, SHARD], BF16, kind="ExternalOutput")

    # collective buffers
    ag_in = nc.dram_tensor("ag_in", [SHARD, HID], BF16)
    table = nc.dram_tensor("table", [NPAD, HID], BF16, addr_space="Shared")
    st_in = nc.dram_tensor("st_in", [HID, 2], F32)
    st_out = nc.dram_tensor("st_out", [NCORES * HID, 2], F32, addr_space="Shared")

    rgroups = [list(range(NCORES))]

    with tile.TileContext(nc) as tc:
        with (
            tc.tile_pool(name="persist", bufs=1) as pp,
            tc.tile_pool(name="work", bufs=3) as wp,
            tc.tile_pool(name="ohp", bufs=12) as ohpool,
            tc.tile_pool(name="psum", bufs=3, space="PSUM") as psp,
            tc.tile_pool(name="psum_tp", bufs=2, space="PSUM") as ptp,
        ):
            nc.gpsimd.load_library(library_config.mlp)

            # ---- persistent loads ----
            idx = pp.tile([128, SUMT * 8], I16)
            nc.sync.dma_start(idx[:], idx_in[:])
            dstrel = pp.tile([128, SUMT], F32)
            nc.sync.dma_start(dstrel[:], dstrel_in[:])
            iota_t = pp.tile([128, 128], BF16)
            nc.sync.dma_start(iota_t[:], iota_in[:])
            w1 = pp.tile([IN_C, HID], BF16)
            nc.sync.dma_start(w1[:], w1_in[:])
            w2 = pp.tile([HID, HID], BF16)
            nc.sync.dma_start(w2[:], w2_in[:])
            w3 = pp.tile([HID, HID], BF16)
            nc.sync.dma_start(w3[:], w3_in[:])
            gb = pp.tile([HID, 6], F32)
            nc.sync.dma_start(gb[:], gb_in[:])
            xT = pp.tile([IN_C, SHARD], BF16)
            nc.sync.dma_start(xT[:], xT_in[:])

            ident = pp.tile([HID, HID], BF16)
            make_identity(nc, ident[:])
            eps_t = pp.tile([HID, 1], F32)
            nc.vector.memset(eps_t[:], float(BN_EPS))

            # dinv in both layouts: rsqrt(deg) = reciprocal(sqrt(deg))
            dinv_bc = pp.tile([HID, SHARD], F32)
            nc.sync.dma_start(dinv_bc[:], degbc_in[:])
            nc.scalar.sqrt(dinv_bc[:], dinv_bc[:])
            nc.vector.reciprocal(dinv_bc[:], dinv_bc[:])

            # persistent activations (channel-major, bf16)
            S_t = pp.tile([HID, SHARD], BF16)    # pre-BN conv output
            X_t = pp.tile([HID, SHARD], BF16)    # post-BN/ReLU activations
            H_t = pp.tile([HID, SHARD], BF16)    # X @ W; reused as stats scratch

            if not do_compute:
                nc.vector.memset(S_t[:], 0.0)

            # pair-row view of the gather table: 25088 rows x 128 bf16 (256B)
            table_pairs = table[:].rearrange("(a b) c -> a (b c)", b=2)

            nreg_cache = {}

            def nreg(v):
                if v not in nreg_cache:
                    nreg_cache[v] = nc.gpsimd.to_reg(v)
                return nreg_cache[v]

            for _rep in range(reps):
              for layer in range(3):
                  w = (w1, w2, w3)[layer]
                  kdim = IN_C if layer == 0 else HID
                  rhs = xT if layer == 0 else X_t

                  # ---- H^T = dinv_src * (W^T @ X^T)  (channel-major) ----
                  col = 0
                  while col < SHARD:
                      nn = min(512, SHARD - col)
                      hp = psp.tile([HID, 512], F32, tag="wmm")
                      nc.tensor.matmul(hp[:, :nn], lhsT=w[:kdim, :], rhs=rhs[:kdim, col:col + nn],
                                       start=True, stop=True)
                      nc.vector.tensor_tensor(out=H_t[:, col:col + nn], in0=hp[:, :nn],
                                              in1=dinv_bc[:, col:col + nn],
                                              op=mybir.AluOpType.mult)
                      col += nn

                  # ---- table shard: transpose to node-major ----
                  for b in range(NB):
                      tp = ptp.tile([128, HID], BF16, tag="tp")
                      nc.tensor.transpose(tp[:], H_t[:, b * 128:(b + 1) * 128], ident[:])
                      nm = wp.tile([128, HID], BF16, tag="nm")
                      nc.vector.tensor_copy(nm[:], tp[:])
                      nc.sync.dma_start(ag_in[b * 128:(b + 1) * 128, :], nm[:])

                  if do_ag:
                      nc.gpsimd.collective_compute(
                          "AllGather", mybir.AluOpType.bypass, replica_groups=rgroups,
                          ins=[ag_in[:]], outs=[table[:]],
                      )

                  # ---- message passing, one gather call per block pair ----
                  qn = 0
                  for grp in groups:
                      gstart, gtiles = grp["call"]
                      msg = wp.tile([128, gtiles * 128], BF16, tag="msg")
                      if not do_gather:
                          nc.vector.memset(msg[0:1, 0:2], 0.0)
                      if do_gather:
                          nc.gpsimd.dma_gather(
                              out_ap=msg[:].rearrange("p (n d) -> p n d", d=128),
                              in_ap=table_pairs,
                              idxs_ap=idx[:, gstart * 8:(gstart + gtiles) * 8],
                              num_idxs=gtiles * 128, num_idxs_reg=nreg(gtiles * 128),
                              elem_size=128, single_packet=False, queue_num=qn % 4,
                          )
                          qn += 1
                      for blk in grp["blocks"]:
                          if not do_compute:
                              continue
                          ps = psp.tile([HID, 128], F32, tag="scat")
                          segs = [(grp["seg"][(blk, 0)], 0),
                                  (grp["seg"][(blk, 1)], 64)]
                          ntot = sum(n for (_, n), _ in segs)
                          ti = 0
                          for (tstart, ntile), choff in segs:
                              for t in range(ntile):
                                  gcol = tstart + t
                                  rel = gcol - gstart
                                  oh = ohpool.tile([128, 128], BF16, tag="oh")
                                  # split one-hot builds across DVE and Pool
                                  # so neither engine's dispatch serializes
                                  # the scatter chain
                                  oh_eng = nc.gpsimd if (gcol % 3 == 2) else nc.vector
                                  oh_eng.tensor_scalar(
                                      out=oh[:], in0=iota_t[:],
                                      scalar1=dstrel[:, gcol:gcol + 1], scalar2=None,
                                      op0=mybir.AluOpType.is_equal)
                                  nc.tensor.matmul(
                                      ps[:],
                                      lhsT=msg[:, rel * 128 + choff:rel * 128 + choff + HID],
                                      rhs=oh[:],
                                      start=(ti == 0), stop=(ti == ntot - 1))
                                  ti += 1
                          nc.vector.tensor_tensor(
                              out=S_t[:, blk * 128:(blk + 1) * 128], in0=ps[:],
                              in1=dinv_bc[:, blk * 128:(blk + 1) * 128],
                              op=mybir.AluOpType.mult)

                  # ---- BN stats (local) ----
                  sums = wp.tile([HID, 2], F32, tag="sums")
                  nc.scalar.activation(H_t[:], S_t[:], mybir.ActivationFunctionType.Identity,
                                       accum_out=sums[:, 0:1])
                  nc.scalar.activation(H_t[:], S_t[:], mybir.ActivationFunctionType.Square,
                                       accum_out=sums[:, 1:2])
                  nc.sync.dma_start(st_in[:], sums[:])
                  nc.gpsimd.collective_compute(
                      "AllGather", mybir.AluOpType.bypass, replica_groups=rgroups,
                      ins=[st_in[:]], outs=[st_out[:]],
                  )
                  # land all 8 partials as [HID, 16] and reduce locally
                  part = wp.tile([HID, 2 * NCORES], F32, tag="part")
                  nc.sync.dma_start(
                      part[:].rearrange("p (c s) -> p c s", c=NCORES),
                      st_out[:].rearrange("(c p) s -> p c s", c=NCORES))
                  gsums = wp.tile([HID, 2], F32, tag="gsums")
                  nc.vector.tensor_tensor(out=part[:, 0:8], in0=part[:, 0:8],
                                          in1=part[:, 8:16],
                                          op=mybir.AluOpType.add)
                  nc.vector.tensor_tensor(out=part[:, 0:4], in0=part[:, 0:4],
                                          in1=part[:, 4:8],
                                          op=mybir.AluOpType.add)
                  nc.vector.tensor_tensor(out=gsums[:], in0=part[:, 0:2],
                                          in1=part[:, 2:4],
                                          op=mybir.AluOpType.add)

                  # mean/var -> A = g*rsqrt(var+eps), B = be - mean*A
                  stat = wp.tile([HID, 4], F32, tag="stat")
                  nc.vector.tensor_scalar(out=stat[:, 0:2], in0=gsums[:], scalar1=1.0 / N,
                                          scalar2=None, op0=mybir.AluOpType.mult)
                  # var = E[x^2] - mean^2
                  nc.vector.tensor_tensor(out=stat[:, 2:3], in0=stat[:, 0:1],
                                          in1=stat[:, 0:1], op=mybir.AluOpType.mult)
                  nc.vector.tensor_tensor(out=stat[:, 2:3], in0=stat[:, 1:2],
                                          in1=stat[:, 2:3], op=mybir.AluOpType.subtract)
                  # sd = sqrt(var + eps); rinv = 1/sd
                  nc.scalar.activation(stat[:, 3:4], stat[:, 2:3],
                                       mybir.ActivationFunctionType.Sqrt, bias=eps_t[:, 0:1])
                  nc.vector.reciprocal(stat[:, 3:4], stat[:, 3:4])
                  ab = wp.tile([HID, 2], F32, tag="ab")
                  nc.vector.tensor_tensor(out=ab[:, 0:1], in0=stat[:, 3:4],
                                          in1=gb[:, 2 * layer:2 * layer + 1],
                                          op=mybir.AluOpType.mult)
                  nc.vector.tensor_tensor(out=ab[:, 1:2], in0=stat[:, 0:1],
                                          in1=ab[:, 0:1], op=mybir.AluOpType.mult)
                  nc.vector.tensor_tensor(out=ab[:, 1:2],
                                          in0=gb[:, 2 * layer + 1:2 * layer + 2],
                                          in1=ab[:, 1:2], op=mybir.AluOpType.subtract)
                  # X = Relu(S*A + B), in halves so the next layer's matmul
                  # can start on columns 0:3136 before the full shard applies
                  HS = SHARD // 2
                  nc.scalar.activation(X_t[:, 0:HS], S_t[:, 0:HS],
                                       mybir.ActivationFunctionType.Relu,
                                       bias=ab[:, 1:2], scale=ab[:, 0:1])
                  nc.scalar.activation(X_t[:, HS:SHARD], S_t[:, HS:SHARD],
                                       mybir.ActivationFunctionType.Relu,
                                       bias=ab[:, 1:2], scale=ab[:, 0:1])

            nc.sync.dma_start(out_t[:], X_t[:])

    _split_multi_waits(nc)
    lower_extended_insts(nc)
    return nc


def _prep(x, edge_index):
    """Host-side integer preprocessing: shard / sort / pad the edge list."""
    src = np.asarray(edge_index[0], dtype=np.int64)
    dst = np.asarray(edge_index[1], dtype=np.int64)
    loops = np.arange(N, dtype=np.int64)
    src = np.concatenate([src, loops])
    dst = np.concatenate([dst, loops])
    deg = np.bincount(dst, minlength=NPAD).astype(np.float32)
    deg[deg == 0] = 1.0

    order = np.argsort(dst, kind="stable")
    src, dst = src[order], dst[order]
    blk = (dst // 128).astype(np.int64)
    # edges grouped per global block; within block split by src parity
    counts = {}
    seg = {}
    bstart = np.searchsorted(blk, np.arange(NPAD // 128 + 1))
    for gb in range(NPAD // 128):
        s, e = bstart[gb], bstart[gb + 1]
        bs, bd = src[s:e], dst[s:e]
        even = (bs & 1) == 0
        seg[gb] = (bs[even] >> 1, bd[even], bs[~even] >> 1, bd[~even])
        counts[gb] = (even.sum(), (~even).sum())

    TA = [0] * NB
    TB = [0] * NB
    for gb in range(NPAD // 128):
        bloc = gb % NB
        ca, cb = counts[gb]
        TA[bloc] = max(TA[bloc], -(-int(ca) // 128))
        TB[bloc] = max(TB[bloc], -(-int(cb) // 128))
    TA = [max(t, 1) for t in TA]
    TB = [max(t, 1) for t in TB]

    groups, SUMT = _plan(TA, TB)
    bf16 = mybir.dt.np(BF16)
    idx_all = np.zeros((NCORES, 128, SUMT * 8), dtype=np.int16)
    dre_all = np.full((NCORES, 128, SUMT), -1.0, dtype=np.float32)
    for c in range(NCORES):
        for grp in groups:
            for blk in grp["blocks"]:
                gb = c * NB + blk
                sa, da, sb, db = seg[gb]
                for parity, (ss, dd) in ((0, (sa, da)), (1, (sb, db))):
                    tstart, T = grp["seg"][(blk, parity)]
                    nslots = T * 128
                    sl_idx = np.zeros(nslots, dtype=np.int16)
                    sl_dre = np.full(nslots, -1.0, dtype=np.float32)
                    k = len(ss)
                    sl_idx[:k] = ss.astype(np.int16)
                    sl_dre[:k] = (dd - gb * 128).astype(np.float32)
                    wr = sl_idx.reshape(nslots // 16, 16).T
                    idx_all[c, :, tstart * 8:(tstart + T) * 8] = np.tile(wr, (8, 1))
                    dre_all[c, :, tstart:tstart + T] = sl_dre.reshape(T, 128).T
    return deg, TA, TB, idx_all, dre_all


_CACHE = {}
_REPS = [1]


def build_and_maps(x, edge_index, w1, b1, g1, be1, w2, b2, g2, be2, w3, b3, g3, be3):
    x = np.asarray(x, dtype=np.float32)
    deg, TA, TB, idx_all, dre_all = _prep(x, edge_index)

    key = (tuple(TA), tuple(TB), _REPS[0])
    if key not in _CACHE:
        _CACHE[key] = build_kernel(TA, TB, reps=_REPS[0])
    nc = _CACHE[key]

    bf16 = mybir.dt.np(BF16)
    xpad = np.zeros((NPAD, IN_C), dtype=np.float32)
    xpad[:N] = x
    iota = np.broadcast_to(np.arange(128, dtype=np.float32), (128, 128))
    in_maps = []
    for c in range(NCORES):
        sl = slice(c * SHARD, (c + 1) * SHARD)
        deg_c = deg[sl]
        in_maps.append({
            "xT": np.ascontiguousarray(xpad[sl].T).astype(bf16),
            "degbc": np.ascontiguousarray(np.broadcast_to(deg_c, (HID, SHARD))),
            "idx": idx_all[c],
            "dstrel": dre_all[c],
            "iota": iota.astype(bf16),
            "w1": np.asarray(w1, dtype=np.float32).astype(bf16),
            "w2": np.asarray(w2, dtype=np.float32).astype(bf16),
            "w3": np.asarray(w3, dtype=np.float32).astype(bf16),
            "gb": np.stack([np.asarray(a, dtype=np.float32)
                            for a in (g1, be1, g2, be2, g3, be3)], axis=1),
        })

    return nc, in_maps


def kernel(**inputs):
    nc, in_maps = build_and_maps(**inputs)
    from concourse.bass_utils import run_bass_kernel_spmd
    res = run_bass_kernel_spmd(nc, in_maps, list(range(NCORES)))
    out = np.concatenate([res.results[c]["outT"].T.astype(np.float32)
                      for c in range(NCORES)], axis=0)
    return np.ascontiguousarray(out[:N])



# revision 12
# speedup vs baseline: 2.1448x; 2.1448x over previous
"""3-layer GCN encoder (GCNConv+BN+ReLU x3) on 8 Trainium2 NeuronCores.

Strategy (graph/data-parallel over destination nodes), v2 = bf16 pipeline:
  - Nodes padded 50000 -> 50176 = 8 * 6272; core c owns dst rows
    [c*6272, (c+1)*6272) = 49 blocks of 128.
  - All activations/weights/messages are bf16 (PSUM accumulates fp32);
    BN statistics and normalization math stay fp32.
  - Per layer l: each core computes its shard of H = (X @ W_l) * dinv
    channel-major on the PE, transposes to node-major bf16, AllGathers
    the full 50176 x 64 bf16 table.
  - The table is addressed as 25088 rows of 128 bf16 (= 256B = two
    nodes per row), so a single int16 index (src >> 1) covers the whole
    node range: no address-half split, and every gather descriptor is a
    256B transfer.
  - Edges (incl. self loops) are sorted by dst block and, within a
    block, partitioned by src parity. A tile of 128 even-src (odd-src)
    edges uses columns 0:64 (64:128) of the gathered pair-rows, so each
    tile needs ONE one-hot [slot -> dstrel] bf16 matmul to segment-sum
    messages into a [64ch x 128dst] PSUM accumulator; the result is
    scaled by dinv[dst] (per-src dinv is folded into the table).
  - BatchNorm: per-channel sum / sumsq over the local shard via ACT
    accum_out, AllReduce across cores, then one fused
    Relu(S*A + B) activation (A = gamma*rsqrt(var+eps), B = beta - mean*A).
    Conv biases are mathematically absorbed by BN's mean subtraction.
  - Host side does integer index preprocessing only (sort/partition/pad,
    degree counting, layout transposes); all FP math runs on device.
"""
import sys
sys.path.insert(0, "/opt/trn_rl_repo")
import numpy as np

import concourse.bass as bass
import concourse.mybir as mybir
import concourse.tile as tile
from concourse import library_config
from concourse.library_overlay import lower_extended_insts
from concourse.masks import make_identity

N = 50000
NPAD = 50176
NCORES = 8
SHARD = NPAD // NCORES          # 6272
NB = SHARD // 128               # 49 blocks per core
NPAIR = NPAD // 2               # 25088 pair-rows (int16-safe)
IN_C = 128
HID = 64
BN_EPS = 1e-5
F32 = mybir.dt.float32
BF16 = mybir.dt.bfloat16
I16 = mybir.dt.int16


def _split_multi_waits(nc, cap=1):
    """walrus in this toolchain accepts one sync wait per instruction;
    hoist extras onto standalone same-engine NOPs."""
    ctr = 0
    for func in nc.m.functions:
        for bb in func.blocks:
            new_insts = []
            for inst in bb.instructions:
                si = inst.sync_info
                if si is not None and len(si.on_wait) > cap:
                    waits = list(si.on_wait)
                    for w in waits[:-cap]:
                        ctr += 1
                        new_insts.append(mybir.InstNoOp(
                            name=f"waitsplit-{ctr}-{inst.name}",
                            sync_info=mybir.SyncInfo(on_wait=[w], on_update=[]),
                            bass_nofuse=True,
                            engine=inst.engine,
                        ))
                    inst.sync_info = mybir.SyncInfo(
                        on_wait=waits[-cap:], on_update=list(si.on_update))
                new_insts.append(inst)
            bb.instructions = new_insts
    return ctr


def _plan(TA, TB):
    """Group blocks in pairs; ONE gather call per group covering the
    paired blocks' even+odd tile segments contiguously.
    Returns (groups, SUMT): groups = list of dicts with
      blocks: tuple of block ids
      call:   (tile_start, ntiles)
      seg:    {(block, parity): (tile_start, ntiles)}
    Tile indices are global (into dstrel / idx column space)."""
    groups = []
    toff = 0
    b = 0
    while b < NB:
        blocks = tuple(range(b, min(b + 4, NB)))
        seg = {}
        call_start = toff
        for blk in blocks:
            seg[(blk, 0)] = (toff, TA[blk])
            toff += TA[blk]
            seg[(blk, 1)] = (toff, TB[blk])
            toff += TB[blk]
        groups.append({
            "blocks": blocks,
            "call": (call_start, toff - call_start),
            "seg": seg,
        })
        b += 4
    return groups, toff


def build_kernel(TA, TB, reps=1, do_gather=True, do_compute=True, do_ag=True):
    """TA/TB: per-block tile counts (len NB) for even-src / odd-src edges,
    uniform across cores (max over cores, baked into the program).
    reps>1 replicates the whole 3-layer body (timing instrument only)."""
    groups, SUMT = _plan(TA, TB)

    nc = bass.Bass(num_swdge_queues=4)
    xT_in = nc.dram_tensor("xT", [IN_C, SHARD], BF16, kind="ExternalInput")
    dinv_in = nc.dram_tensor("dinv", [128, NB], F32, kind="ExternalInput")
    idx_in = nc.dram_tensor("idx", [128, SUMT * 8], I16, kind="ExternalInput")
    dstrel_in = nc.dram_tensor("dstrel", [128, SUMT], F32, kind="ExternalInput")
    iota_in = nc.dram_tensor("iota", [128, 128], BF16, kind="ExternalInput")
    w1_in = nc.dram_tensor("w1", [IN_C, HID], BF16, kind="ExternalInput")
    w2_in = nc.dram_tensor("w2", [HID, HID], BF16, kind="ExternalInput")
    w3_in = nc.dram_tensor("w3", [HID, HID], BF16, kind="ExternalInput")
    gb_in = nc.dram_tensor("gb", [HID, 6], F32, kind="ExternalInput")  # g1,be1,g2,be2,g3,be3
    out_t = nc.dram_tensor("outT", [HID, SHARD], BF16, kind="ExternalOutput")

    # collective buffers
    ag_in = nc.dram_tensor("ag_in", [SHARD, HID], BF16)
    table = nc.dram_tensor("table", [NPAD, HID], BF16, addr_space="Shared")
    st_in = nc.dram_tensor("st_in", [HID, 2], F32)
    st_out = nc.dram_tensor("st_out", [NCORES * HID, 2], F32, addr_space="Shared")

    rgroups = [list(range(NCORES))]

    with tile.TileContext(nc) as tc:
        with (
            tc.tile_pool(name="persist", bufs=1) as pp,
            tc.tile_pool(name="work", bufs=3) as wp,
            tc.tile_pool(name="ohp", bufs=12) as ohpool,
            tc.tile_pool(name="psum", bufs=3, space="PSUM") as psp,
            tc.tile_pool(name="psum_mm", bufs=2, space="PSUM") as pmm,
            tc.tile_pool(name="psum_tp", bufs=2, space="PSUM") as ptp,
        ):
            nc.gpsimd.load_library(library_config.mlp)

            # ---- persistent loads ----
            idx = pp.tile([128, SUMT * 8], I16)
            nc.sync.dma_start(idx[:], idx_in[:])
            dstrel = pp.tile([128, SUMT], F32)
            nc.sync.dma_start(dstrel[:], dstrel_in[:])
            iota_t = pp.tile([128, 128], BF16)
            nc.sync.dma_start(iota_t[:], iota_in[:])
            w1 = pp.tile([IN_C, HID], BF16)
            nc.sync.dma_start(w1[:], w1_in[:])
            w2 = pp.tile([HID, HID], BF16)
            nc.sync.dma_start(w2[:], w2_in[:])
            w3 = pp.tile([HID, HID], BF16)
            nc.sync.dma_start(w3[:], w3_in[:])
            gb = pp.tile([HID, 6], F32)
            nc.sync.dma_start(gb[:], gb_in[:])
            xT = pp.tile([IN_C, SHARD], BF16)
            nc.sync.dma_start(xT[:], xT_in[:])

            ident = pp.tile([HID, HID], BF16)
            make_identity(nc, ident[:])
            ident128 = pp.tile([128, 128], BF16)
            make_identity(nc, ident128[:])
            eps_t = pp.tile([HID, 1], F32)
            nc.vector.memset(eps_t[:], float(BN_EPS))

            # dinv node-major [128, NB]: rsqrt(deg), zeroed for padding nodes
            # (zeroes their table rows -> exact BN stats)
            dinv_nm = pp.tile([128, NB], F32)
            nc.sync.dma_start(dinv_nm[:], dinv_in[:])

            # persistent activations (channel-major, bf16)
            S_t = pp.tile([HID, SHARD], BF16)    # pre-BN conv output
            X_t = pp.tile([HID, SHARD], BF16)    # post-BN/ReLU activations
            H_t = pp.tile([HID, SHARD], BF16)    # X @ W; reused as stats scratch

            if not do_compute:
                nc.vector.memset(S_t[:], 0.0)

            # pair-row view of the gather table: 25088 rows x 128 bf16 (256B)
            table_pairs = table[:].rearrange("(a b) c -> a (b c)", b=2)

            nreg_cache = {}

            def nreg(v):
                if v not in nreg_cache:
                    nreg_cache[v] = nc.gpsimd.to_reg(v)
                return nreg_cache[v]

            for _rep in range(reps):
              for layer in range(3):
                  w = (w1, w2, w3)[layer]
                  kdim = IN_C if layer == 0 else HID
                  rhs = xT if layer == 0 else X_t

                  # ---- H^T = W^T @ X^T  (channel-major; dinv folded later) ----
                  col = 0
                  while col < SHARD:
                      nn = min(512, SHARD - col)
                      hp = pmm.tile([HID, 512], F32, tag="wmm")
                      nc.tensor.matmul(hp[:, :nn], lhsT=w[:kdim, :], rhs=rhs[:kdim, col:col + nn],
                                       start=True, stop=True)
                      nc.scalar.copy(H_t[:, col:col + nn], hp[:, :nn])
                      col += nn

                  # ---- table shard: transpose to node-major, fold dinv_src
                  # (per-partition scale on the Activation engine) ----
                  for b in range(NB):
                      tp = ptp.tile([128, HID], BF16, tag="tp")
                      nc.tensor.transpose(tp[:], H_t[:, b * 128:(b + 1) * 128], ident[:])
                      nm = wp.tile([128, HID], BF16, tag="nm")
                      nc.scalar.activation(nm[:], tp[:],
                                           mybir.ActivationFunctionType.Identity,
                                           scale=dinv_nm[:, b:b + 1])
                      nc.sync.dma_start(ag_in[b * 128:(b + 1) * 128, :], nm[:])

                  if do_ag:
                      nc.gpsimd.collective_compute(
                          "AllGather", mybir.AluOpType.bypass, replica_groups=rgroups,
                          ins=[ag_in[:]], outs=[table[:]],
                      )

                  # ---- message passing, one gather call per block pair ----
                  qn = 0
                  for grp in groups:
                      gstart, gtiles = grp["call"]
                      msg = wp.tile([128, gtiles * 128], BF16, tag="msg")
                      if not do_gather:
                          nc.vector.memset(msg[0:1, 0:2], 0.0)
                      if do_gather:
                          nc.gpsimd.dma_gather(
                              out_ap=msg[:].rearrange("p (n d) -> p n d", d=128),
                              in_ap=table_pairs,
                              idxs_ap=idx[:, gstart * 8:(gstart + gtiles) * 8],
                              num_idxs=gtiles * 128, num_idxs_reg=nreg(gtiles * 128),
                              elem_size=128, single_packet=False, queue_num=qn % 4,
                          )
                          qn += 1
                      for blk in grp["blocks"]:
                          if not do_compute:
                              continue
                          # out[128dst, 64ch]: one-hot as stationary so the
                          # dinv_dst scale is a per-partition Activation op —
                          # the DVE queue carries ONLY one-hots and never
                          # stalls on PE (keeps gathers pipelined).
                          ps = psp.tile([128, HID], F32, tag="scat", name="ps")
                          segs = [(grp["seg"][(blk, 0)], 0),
                                  (grp["seg"][(blk, 1)], 64)]
                          ntot = sum(n for (_, n), _ in segs)
                          ti = 0
                          for (tstart, ntile), choff in segs:
                              for t in range(ntile):
                                  gcol = tstart + t
                                  rel = gcol - gstart
                                  oh = ohpool.tile([128, 128], BF16, tag="oh")
                                  nc.vector.tensor_scalar(
                                      out=oh[:], in0=iota_t[:],
                                      scalar1=dstrel[:, gcol:gcol + 1], scalar2=None,
                                      op0=mybir.AluOpType.is_equal)
                                  nc.tensor.matmul(
                                      ps[:],
                                      lhsT=oh[:],
                                      rhs=msg[:, rel * 128 + choff:rel * 128 + choff + HID],
                                      start=(ti == 0), stop=(ti == ntot - 1))
                                  ti += 1
                          # dinv_dst scale + cast (ACT), node-major
                          sn = wp.tile([128, HID], BF16, tag="sn")
                          nc.scalar.activation(
                              sn[:], ps[:],
                              mybir.ActivationFunctionType.Identity,
                              scale=dinv_nm[:, blk:blk + 1])
                          # transpose to channel-major into S_t (PE + ACT)
                          tps = ptp.tile([HID, 128], BF16, tag="tps", bufs=1)
                          nc.tensor.transpose(tps[:], sn[:], ident128[:])
                          nc.scalar.copy(S_t[:, blk * 128:(blk + 1) * 128], tps[:])

                  # ---- BN stats (local) ----
                  sums = wp.tile([HID, 2], F32, tag="sums")
                  nc.scalar.activation(H_t[:], S_t[:], mybir.ActivationFunctionType.Identity,
                                       accum_out=sums[:, 0:1])
                  nc.scalar.activation(H_t[:], S_t[:], mybir.ActivationFunctionType.Square,
                                       accum_out=sums[:, 1:2])
                  nc.sync.dma_start(st_in[:], sums[:])
                  nc.gpsimd.collective_compute(
                      "AllGather", mybir.AluOpType.bypass, replica_groups=rgroups,
                      ins=[st_in[:]], outs=[st_out[:]],
                  )
                  # land all 8 partials as [HID, 16] and reduce locally
                  part = wp.tile([HID, 2 * NCORES], F32, tag="part")
                  nc.sync.dma_start(
                      part[:].rearrange("p (c s) -> p c s", c=NCORES),
                      st_out[:].rearrange("(c p) s -> p c s", c=NCORES))
                  gsums = wp.tile([HID, 2], F32, tag="gsums")
                  nc.vector.tensor_tensor(out=part[:, 0:8], in0=part[:, 0:8],
                                          in1=part[:, 8:16],
                                          op=mybir.AluOpType.add)
                  nc.vector.tensor_tensor(out=part[:, 0:4], in0=part[:, 0:4],
                                          in1=part[:, 4:8],
                                          op=mybir.AluOpType.add)
                  nc.vector.tensor_tensor(out=gsums[:], in0=part[:, 0:2],
                                          in1=part[:, 2:4],
                                          op=mybir.AluOpType.add)

                  # mean/var -> A = g*rsqrt(var+eps), B = be - mean*A
                  stat = wp.tile([HID, 4], F32, tag="stat")
                  nc.vector.tensor_scalar(out=stat[:, 0:2], in0=gsums[:], scalar1=1.0 / N,
                                          scalar2=None, op0=mybir.AluOpType.mult)
                  # var = E[x^2] - mean^2
                  nc.vector.tensor_tensor(out=stat[:, 2:3], in0=stat[:, 0:1],
                                          in1=stat[:, 0:1], op=mybir.AluOpType.mult)
                  nc.vector.tensor_tensor(out=stat[:, 2:3], in0=stat[:, 1:2],
                                          in1=stat[:, 2:3], op=mybir.AluOpType.subtract)
                  # sd = sqrt(var + eps); rinv = 1/sd
                  nc.scalar.activation(stat[:, 3:4], stat[:, 2:3],
                                       mybir.ActivationFunctionType.Sqrt, bias=eps_t[:, 0:1])
                  nc.vector.reciprocal(stat[:, 3:4], stat[:, 3:4])
                  ab = wp.tile([HID, 2], F32, tag="ab")
                  nc.vector.tensor_tensor(out=ab[:, 0:1], in0=stat[:, 3:4],
                                          in1=gb[:, 2 * layer:2 * layer + 1],
                                          op=mybir.AluOpType.mult)
                  nc.vector.tensor_tensor(out=ab[:, 1:2], in0=stat[:, 0:1],
                                          in1=ab[:, 0:1], op=mybir.AluOpType.mult)
                  nc.vector.tensor_tensor(out=ab[:, 1:2],
                                          in0=gb[:, 2 * layer + 1:2 * layer + 2],
                                          in1=ab[:, 1:2], op=mybir.AluOpType.subtract)
                  # X = Relu(S*A + B), in halves so the next layer's matmul
                  # can start on columns 0:3136 before the full shard applies
                  HS = SHARD // 2
                  nc.scalar.activation(X_t[:, 0:HS], S_t[:, 0:HS],
                                       mybir.ActivationFunctionType.Relu,
                                       bias=ab[:, 1:2], scale=ab[:, 0:1])
                  nc.scalar.activation(X_t[:, HS:SHARD], S_t[:, HS:SHARD],
                                       mybir.ActivationFunctionType.Relu,
                                       bias=ab[:, 1:2], scale=ab[:, 0:1])

            nc.sync.dma_start(out_t[:], X_t[:])

    _split_multi_waits(nc)
    lower_extended_insts(nc)
    return nc


def _prep(x, edge_index):
    """Host-side integer preprocessing: shard / sort / pad the edge list."""
    src = np.asarray(edge_index[0], dtype=np.int64)
    dst = np.asarray(edge_index[1], dtype=np.int64)
    loops = np.arange(N, dtype=np.int64)
    src = np.concatenate([src, loops])
    dst = np.concatenate([dst, loops])
    deg = np.bincount(dst, minlength=NPAD).astype(np.float32)
    deg[deg == 0] = 1.0
    dinv = (1.0 / np.sqrt(deg)).astype(np.float32)
    dinv[N:] = 0.0  # zero padding nodes' table rows -> exact BN stats

    order = np.argsort(dst, kind="stable")
    src, dst = src[order], dst[order]
    blk = (dst // 128).astype(np.int64)
    # edges grouped per global block; within block split by src parity
    counts = {}
    seg = {}
    bstart = np.searchsorted(blk, np.arange(NPAD // 128 + 1))
    for gb in range(NPAD // 128):
        s, e = bstart[gb], bstart[gb + 1]
        bs, bd = src[s:e], dst[s:e]
        even = (bs & 1) == 0
        seg[gb] = (bs[even] >> 1, bd[even], bs[~even] >> 1, bd[~even])
        counts[gb] = (even.sum(), (~even).sum())

    TA = [0] * NB
    TB = [0] * NB
    for gb in range(NPAD // 128):
        bloc = gb % NB
        ca, cb = counts[gb]
        TA[bloc] = max(TA[bloc], -(-int(ca) // 128))
        TB[bloc] = max(TB[bloc], -(-int(cb) // 128))
    TA = [max(t, 1) for t in TA]
    TB = [max(t, 1) for t in TB]

    groups, SUMT = _plan(TA, TB)
    bf16 = mybir.dt.np(BF16)
    idx_all = np.zeros((NCORES, 128, SUMT * 8), dtype=np.int16)
    dre_all = np.full((NCORES, 128, SUMT), -1.0, dtype=np.float32)
    for c in range(NCORES):
        for grp in groups:
            for blk in grp["blocks"]:
                gb = c * NB + blk
                sa, da, sb, db = seg[gb]
                for parity, (ss, dd) in ((0, (sa, da)), (1, (sb, db))):
                    tstart, T = grp["seg"][(blk, parity)]
                    nslots = T * 128
                    sl_idx = np.zeros(nslots, dtype=np.int16)
                    sl_dre = np.full(nslots, -1.0, dtype=np.float32)
                    k = len(ss)
                    sl_idx[:k] = ss.astype(np.int16)
                    sl_dre[:k] = (dd - gb * 128).astype(np.float32)
                    wr = sl_idx.reshape(nslots // 16, 16).T
                    idx_all[c, :, tstart * 8:(tstart + T) * 8] = np.tile(wr, (8, 1))
                    dre_all[c, :, tstart:tstart + T] = sl_dre.reshape(T, 128).T
    return dinv, TA, TB, idx_all, dre_all


_CACHE = {}
_REPS = [1]


def build_and_maps(x, edge_index, w1, b1, g1, be1, w2, b2, g2, be2, w3, b3, g3, be3):
    x = np.asarray(x, dtype=np.float32)
    dinv, TA, TB, idx_all, dre_all = _prep(x, edge_index)

    key = (tuple(TA), tuple(TB), _REPS[0])
    if key not in _CACHE:
        _CACHE[key] = build_kernel(TA, TB, reps=_REPS[0])
    nc = _CACHE[key]

    bf16 = mybir.dt.np(BF16)
    xpad = np.zeros((NPAD, IN_C), dtype=np.float32)
    xpad[:N] = x
    iota = np.broadcast_to(np.arange(128, dtype=np.float32), (128, 128))
    in_maps = []
    for c in range(NCORES):
        sl = slice(c * SHARD, (c + 1) * SHARD)
        in_maps.append({
            "xT": np.ascontiguousarray(xpad[sl].T).astype(bf16),
            "dinv": np.ascontiguousarray(dinv[sl].reshape(NB, 128).T),
            "idx": idx_all[c],
            "dstrel": dre_all[c],
            "iota": iota.astype(bf16),
            "w1": np.asarray(w1, dtype=np.float32).astype(bf16),
            "w2": np.asarray(w2, dtype=np.float32).astype(bf16),
            "w3": np.asarray(w3, dtype=np.float32).astype(bf16),
            "gb": np.stack([np.asarray(a, dtype=np.float32)
                            for a in (g1, be1, g2, be2, g3, be3)], axis=1),
        })

    return nc, in_maps


def kernel(**inputs):
    nc, in_maps = build_and_maps(**inputs)
    from concourse.bass_utils import run_bass_kernel_spmd
    res = run_bass_kernel_spmd(nc, in_maps, list(range(NCORES)))
    out = np.concatenate([res.results[c]["outT"].T.astype(np.float32)
                      for c in range(NCORES)], axis=0)
    return np.ascontiguousarray(out[:N])

